# revision 23
# baseline (speedup 1.0000x reference)
"""ColBERT MaxSim loss kernel for Trainium2 (8 NeuronCores).

Strategy: shard docs c 8-way (64 docs/core). The device computes a SCREEN
of the per-(query-token, doc) MaxSim values m[t,c] = max_d q_t.p_cd; the
host sums over s, picks top-K candidate docs per query plus doc 0,
recomputes those few exactly in fp64, and evaluates the loss. Device
numerics only have to rank docs (true top-1 gaps are ~hundreds vs screen
noise of a few units), so fp8 matmuls are safe.

Device bottleneck is PSUM egress (only DVE and ACT can read PSUM, 1
col/cycle each; tensor_tensor cannot take two PSUM operands and Pool/DMA
cannot touch PSUM). The 65536 psum columns per core are drained by three
lanes sized to finish together, each with its own 4-bank psum pool so no
lane ever blocks another:
  - TD docs, token-partition layout: DVE tensor_reduce(max) PSUM->m direct.
  - PD docs, d-partition layout: ACT copy PSUM->fp16 SBUF, then Pool
    partition_all_reduce(max) finishes each doc in one op; row 0 shipped.
  - SD docs, d-partition layout: ACT copy PSUM->fp16 SBUF, DMA ships raw
    tiles to DRAM; the host takes the max over d (free ns on idle DMA).
Matmuls are fp8e4m3 DoubleRow (0.5 cycles/row): PE stays under the drain
even when psum backpressure keeps it at the mid p-state clock.
"""

import numpy as np

import concourse.bacc as bacc
import concourse.bass as bass
import concourse.tile as tile
from concourse import bass_isa, mybir
from concourse.bass_utils import run_bass_kernel_spmd

N_CORES = 8
B, S, H = 32, 32, 128
C, D = 512, 128
C_LOC = C // N_CORES          # 64 docs per core
T = B * S                     # 1024 query tokens
N_TCHUNK = T // 128           # 8 chunks of 128 tokens
TEMPERATURE = 0.02
K_RESCORE = 32                # exact-rescore candidates per query

# doc-class sizes per core (sum = C_LOC)
TD = 31                       # DVE direct-reduce docs (token layout)
PD = 21                       # ACT+Pool docs (d layout)
SD = C_LOC - TD - PD          # ACT+DMA ship docs (d layout)

MM_DTYPE = "float8dr"         # kept for test.py compat
FRONT_NUM, FRONT_DEN = 8, 7   # d-doc emission front-loading ratio

_NC_CACHE = {}
LAST_RESULTS = None

N_PPAIR = (PD + 1) // 2       # pool ops cover 2 docs (last may be 1)
N_SPAIR = (SD + 1) // 2       # ship DMAs cover 2 docs (last may be 1)


def _tgroups():
    """T-doc groups per chunk (psum tiles of <=8 docs)."""
    out = []
    j = 0
    while j < TD:
        n = min(8, TD - j)
        out.append((j, n))
        j += n
    return out


def _build(mode: str) -> bass.Bass:
    assert mode == "float8dr"
    f16 = mybir.dt.float16
    f32 = mybir.dt.float32
    f8 = mybir.dt.float8e4
    mx = mybir.AluOpType.max
    DR = mybir.MatmulPerfMode.DoubleRow

    nc = bacc.Bacc(None, target_bir_lowering=False)
    qT = nc.dram_tensor("qT", [H // 2, 2, T], f8, kind="ExternalInput")
    pT = nc.dram_tensor("pT", [H // 2, 2, C_LOC * D], f8, kind="ExternalInput")
    m_out = nc.dram_tensor("m_out", [N_TCHUNK, 128, TD], f32,
                           kind="ExternalOutput")
    pool_out_d = nc.dram_tensor("pool_row", [N_PPAIR * 2048], f16,
                                kind="ExternalOutput")
    ship_out = nc.dram_tensor("ship_out", [N_SPAIR, 128, 2048], f16,
                              kind="ExternalOutput")

    tg = _tgroups()

    with tile.TileContext(nc) as tc:
        with (
            tc.tile_pool(name="consts", bufs=1) as consts,
            tc.tile_pool(name="ps", bufs=4, space="PSUM") as psum_pool,
            tc.tile_pool(name="pin", bufs=4) as pin_pool,
            tc.tile_pool(name="shp", bufs=4) as shp_pool,
            tc.tile_pool(name="mtp", bufs=4) as mt_pool,
        ):
            qT_sb = consts.tile([H // 2, 2, T], f8)
            nc.sync.dma_start(out=qT_sb, in_=qT[:, :, :])
            pT_sb = consts.tile([H // 2, 2, C_LOC * D], f8)
            # pieces ordered so the first T-docs and first pool docs
            # arrive first (both lanes start early)
            pieces = [(0, 8), (TD, TD + 8), (8, 16), (TD + 8, TD + 16),
                      (16, 24), (TD + 16, C_LOC), (24, TD)]
            for (a, b) in pieces:
                sl = slice(a * D, b * D)
                nc.sync.dma_start(out=pT_sb[:, :, sl], in_=pT[:, :, sl])

            pool_res = consts.tile([128, N_PPAIR * 2048], f16)

            # d-layout docs: pool pairs front-loaded (Pool engine trails its
            # ACT feed by ~1.5x), ship pairs interleaved behind them
            pool_l = [("pool", j, TD + j) for j in range(PD)]
            ship_l = [("ship", j, TD + PD + j) for j in range(SD)]
            ddocs = []
            pi = si = 0
            while pi < PD or si < SD:
                take_p = min(4 if pi else 6, PD - pi)
                ddocs += pool_l[pi:pi + take_p]; pi += take_p
                take_s = min(2, SD - si)
                ddocs += ship_l[si:si + take_s]; si += take_s
            state = {"emitted": 0, "pin": None, "shp": None}

            def emit_ddoc():
                i = state["emitted"]
                if i >= len(ddocs):
                    return
                state["emitted"] = i + 1
                kind, idx, doc = ddocs[i]
                half = idx % 2
                ps = psum_pool.tile([128, 1024], f32, tag="ps")
                lhs = pT_sb[:, :, doc * D:(doc + 1) * D]
                nc.tensor.matmul(ps[:, 0:512], lhs, qT_sb[:, :, 0:512],
                                 start=True, stop=True, perf_mode=DR)
                nc.tensor.matmul(ps[:, 512:1024], lhs, qT_sb[:, :, 512:1024],
                                 start=True, stop=True, perf_mode=DR)
                if kind == "pool":
                    if half == 0:
                        state["pin"] = pin_pool.tile([128, 2048], f16,
                                                     name="pin_t", tag="pin")
                    pin = state["pin"]
                    nc.scalar.copy(out=pin[:, half * 1024:(half + 1) * 1024],
                                   in_=ps)
                    last = idx == PD - 1
                    if half == 1 or last:
                        pair = idx // 2
                        w = 1024 if (last and half == 0) else 2048
                        nc.gpsimd.partition_all_reduce(
                            pool_res[:, pair * 2048:pair * 2048 + w],
                            pin[:, 0:w], channels=128,
                            reduce_op=bass_isa.ReduceOp.max)
                        if pair == 5:
                            nc.sync.dma_start(
                                out=pool_out_d[0:6 * 2048],
                                in_=pool_res[0:1, 0:6 * 2048])
                else:
                    if half == 0:
                        state["shp"] = shp_pool.tile([128, 2048], f16,
                                                     name="shp_t", tag="shp")
                    sc = state["shp"]
                    nc.scalar.copy(out=sc[:, half * 1024:(half + 1) * 1024],
                                   in_=ps)
                    last = idx == SD - 1
                    if half == 1 or last:
                        nc.sync.dma_start(out=ship_out[idx // 2], in_=sc)

            n_dtiles = len(ddocs)
            for k in range(N_TCHUNK):
                q_chunk = qT_sb[:, :, k * 128:(k + 1) * 128]
                m_t = mt_pool.tile([128, TD], f32, name="m_t", tag="mt")
                for gi, (j0, n) in enumerate(tg):
                    ps = psum_pool.tile([128, 1024], f32, tag="ps")
                    ncols = n * D
                    i = 0
                    while i < ncols:
                        w = min(512, ncols - i)
                        nc.tensor.matmul(
                            ps[:, i:i + w], q_chunk,
                            pT_sb[:, :, j0 * D + i:j0 * D + i + w],
                            start=True, stop=True, perf_mode=DR)
                        i += w
                    nc.vector.tensor_reduce(
                        out=m_t[:, j0:j0 + n],
                        in_=ps[:, 0:ncols].rearrange("p (g d) -> p g d", d=D),
                        axis=mybir.AxisListType.X, op=mx)
                    if j0 + n >= TD:
                        nc.sync.dma_start(out=m_out[k], in_=m_t)
                    # interleave d-layout docs between T-tiles; finish them
                    # by ~7/8 of the chunks so pool/ship drain during the
                    # last chunk's DVE work
                    step = k * len(tg) + gi + 1
                    want = (n_dtiles * FRONT_NUM * step) // (
                        FRONT_DEN * len(tg) * N_TCHUNK)
                    while state["emitted"] < min(want, n_dtiles):
                        emit_ddoc()

            nc.sync.dma_start(out=pool_out_d[6 * 2048:],
                              in_=pool_res[0:1, 6 * 2048:])
    nc.compile()
    return nc


def _get_nc(mode: str = "float8dr") -> bass.Bass:
    if mode not in _NC_CACHE:
        _NC_CACHE[mode] = _build(mode)
    return _NC_CACHE[mode]


def kernel(query_embeddings, positive_embeddings):
    global LAST_RESULTS
    q = np.ascontiguousarray(np.asarray(query_embeddings, dtype=np.float32))
    p = np.ascontiguousarray(np.asarray(positive_embeddings, dtype=np.float32))
    assert q.shape == (B, S, H) and p.shape == (C, D, H)

    import ml_dtypes
    f8 = ml_dtypes.float8_e4m3
    # h split over [64 partitions, 2 k-subtiles]: h = 2*p + j
    qT8 = np.ascontiguousarray(
        q.reshape(T, H).T.reshape(H // 2, 2, T)).astype(f8)
    pT = p.transpose(2, 0, 1)                                  # [H, C, D] view
    in_maps = []
    for core in range(N_CORES):
        blk = pT[:, core * C_LOC:(core + 1) * C_LOC, :]        # [H, C_LOC, D]
        in_maps.append({
            "qT": qT8,
            "pT": np.ascontiguousarray(
                blk.reshape(H // 2, 2, C_LOC * D)).astype(f8),
        })

    nc = _get_nc(MM_DTYPE)
    res = run_bass_kernel_spmd(
        nc, in_maps, core_ids=list(range(N_CORES)), trace=False)
    LAST_RESULTS = res

    m_parts = []
    for core, r in enumerate(res.results):
        m = np.empty((T, C_LOC), dtype=np.float32)
        # DVE lane: m_out [128, k, j] -> token = k*128 + p
        mt = r["m_out"]                                        # [k, 128, TD]
        for k in range(N_TCHUNK):
            m[k * 128:(k + 1) * 128, 0:TD] = mt[k]
        # pool lane: row [N_PPAIR, 2, 1024] -> doc TD + 2t+h, token col
        pr = r["pool_row"].astype(np.float32).reshape(N_PPAIR, 2, 1024)
        for j in range(PD):
            m[:, TD + j] = pr[j // 2, j % 2]
        # ship lane: [N_SPAIR, 128, 2048] -> max over d (axis 1)
        sh = r["ship_out"].astype(np.float32).max(axis=1)      # [N_SPAIR,2048]
        sh = sh.reshape(N_SPAIR, 2, 1024)
        for j in range(SD):
            m[:, TD + PD + j] = sh[j // 2, j % 2]
        m_parts.append(m)
    m_full = np.concatenate(m_parts, axis=1)                   # [T, C]

    screen = m_full.reshape(B, S, C).sum(axis=1, dtype=np.float64)
    screen /= TEMPERATURE                                      # [B, C]

    # exact rescore of top-K candidates (plus doc 0) per query, fp64
    loss_terms = np.empty(B, dtype=np.float64)
    q64 = q.astype(np.float64)
    p64 = p.astype(np.float64)
    def exact_scores(b, cand):
        late = np.einsum("sh,cdh->csd", q64[b], p64[cand])     # [k, S, D]
        return late.max(axis=2).sum(axis=1) / TEMPERATURE      # [k]

    for b in range(B):
        cand = np.argsort(screen[b])[-K_RESCORE:]
        cand = np.unique(np.concatenate([cand, [0]]))
        s_exact = exact_scores(b, cand)
        sb = screen[b].copy()
        sb[cand] = s_exact
        # guard: exact-rescore anything screened within 500 of the exact
        # candidate max (normally selects nothing; catches screen glitches)
        thresh = s_exact.max() - 500.0
        extra = np.setdiff1d(np.where(sb > thresh)[0], cand)
        if extra.size:
            sb[extra] = exact_scores(b, extra)
        mxv = sb.max()
        lse = mxv + np.log(np.exp(sb - mxv).sum())
        loss_terms[b] = lse - sb[0]
    loss = loss_terms.mean()
    return np.asarray(loss, dtype=np.float32)


# revision 24
# speedup vs baseline: 1.0037x; 1.0037x over previous
"""ColBERT MaxSim loss kernel for Trainium2 (8 NeuronCores).

Strategy: shard docs c 8-way (64 docs/core). The device computes a SCREEN
of the per-(query-token, doc) MaxSim values m[t,c] = max_d q_t.p_cd; the
host sums over s, picks top-K candidate docs per query plus doc 0,
recomputes those few exactly in fp64, and evaluates the loss. Device
numerics only have to rank docs (true top-1 gaps are ~hundreds vs screen
noise of a few units), so fp8 matmuls are safe.

Device bottleneck is PSUM egress (only DVE and ACT can read PSUM, 1
col/cycle each; tensor_tensor cannot take two PSUM operands and Pool/DMA
cannot touch PSUM). The 65536 psum columns per core are drained by three
lanes sized to finish together, each with its own 4-bank psum pool so no
lane ever blocks another:
  - TD docs, token-partition layout: DVE tensor_reduce(max) PSUM->m direct.
  - PD docs, d-partition layout: ACT copy PSUM->fp16 SBUF, then Pool
    partition_all_reduce(max) finishes each doc in one op; row 0 shipped.
  - SD docs, d-partition layout: ACT copy PSUM->fp16 SBUF, DMA ships raw
    tiles to DRAM; the host takes the max over d (free ns on idle DMA).
Matmuls are fp8e4m3 DoubleRow (0.5 cycles/row): PE stays under the drain
even when psum backpressure keeps it at the mid p-state clock.
"""

import numpy as np

import concourse.bacc as bacc
import concourse.bass as bass
import concourse.tile as tile
from concourse import bass_isa, mybir
from concourse.bass_utils import run_bass_kernel_spmd

N_CORES = 8
B, S, H = 32, 32, 128
C, D = 512, 128
C_LOC = C // N_CORES          # 64 docs per core
T = B * S                     # 1024 query tokens
N_TCHUNK = T // 128           # 8 chunks of 128 tokens
TEMPERATURE = 0.02
K_RESCORE = 32                # exact-rescore candidates per query

# doc-class sizes per core (sum = C_LOC)
TD = 31                       # DVE direct-reduce docs (token layout)
PD = 21                       # ACT+Pool docs (d layout)
SD = C_LOC - TD - PD          # ACT+DMA ship docs (d layout)

MM_DTYPE = "float8dr"         # kept for test.py compat
FRONT_NUM, FRONT_DEN = 8, 7   # d-doc emission front-loading ratio

_NC_CACHE = {}
LAST_RESULTS = None

N_PPAIR = (PD + 1) // 2       # pool ops cover 2 docs (last may be 1)
N_SPAIR = (SD + 1) // 2       # ship DMAs cover 2 docs (last may be 1)


def _tgroups():
    """T-doc groups per chunk (psum tiles of <=8 docs)."""
    out = []
    j = 0
    while j < TD:
        n = min(8, TD - j)
        out.append((j, n))
        j += n
    return out


def _build(mode: str) -> bass.Bass:
    assert mode == "float8dr"
    f16 = mybir.dt.float16
    f32 = mybir.dt.float32
    f8 = mybir.dt.float8e4
    mx = mybir.AluOpType.max
    DR = mybir.MatmulPerfMode.DoubleRow

    nc = bacc.Bacc(None, target_bir_lowering=False)
    qp = nc.dram_tensor("qp", [H // 2, 2, T + C_LOC * D], f8,
                        kind="ExternalInput")
    m_out = nc.dram_tensor("m_out", [N_TCHUNK, 128, TD], f32,
                           kind="ExternalOutput")
    pool_out_d = nc.dram_tensor("pool_row", [N_PPAIR * 2048], f16,
                                kind="ExternalOutput")
    ship_out = nc.dram_tensor("ship_out", [N_SPAIR, 128, 2048], f16,
                              kind="ExternalOutput")

    tg = _tgroups()

    with tile.TileContext(nc) as tc:
        with (
            tc.tile_pool(name="consts", bufs=1) as consts,
            tc.tile_pool(name="ps", bufs=4, space="PSUM") as psum_pool,
            tc.tile_pool(name="pin", bufs=4) as pin_pool,
            tc.tile_pool(name="shp", bufs=4) as shp_pool,
            tc.tile_pool(name="mtp", bufs=4) as mt_pool,
        ):
            qp_sb = consts.tile([H // 2, 2, T + C_LOC * D], f8)
            qT_sb = qp_sb
            # first piece carries q AND the first T-docs in ONE dma chain;
            # later pieces ordered so pool docs arrive early too
            pieces = [(-T, 8), (TD, TD + 8), (8, 16), (TD + 8, TD + 16),
                      (16, 24), (TD + 16, C_LOC), (24, TD)]
            for (a, b) in pieces:
                sl = slice(T + a * D if a >= 0 else 0, T + b * D)
                nc.sync.dma_start(out=qp_sb[:, :, sl], in_=qp[:, :, sl])

            pool_res = consts.tile([128, N_PPAIR * 2048], f16)

            # d-layout docs: pool pairs front-loaded (Pool engine trails its
            # ACT feed by ~1.5x), ship pairs interleaved behind them
            pool_l = [("pool", j, TD + j) for j in range(PD)]
            ship_l = [("ship", j, TD + PD + j) for j in range(SD)]
            ddocs = []
            pi = si = 0
            while pi < PD or si < SD:
                take_p = min(4 if pi else 6, PD - pi)
                ddocs += pool_l[pi:pi + take_p]; pi += take_p
                take_s = min(2, SD - si)
                ddocs += ship_l[si:si + take_s]; si += take_s
            state = {"emitted": 0, "pin": None, "shp": None}

            def emit_ddoc():
                i = state["emitted"]
                if i >= len(ddocs):
                    return
                state["emitted"] = i + 1
                kind, idx, doc = ddocs[i]
                half = idx % 2
                ps = psum_pool.tile([128, 1024], f32, tag="ps")
                lhs = qp_sb[:, :, T + doc * D:T + (doc + 1) * D]
                nc.tensor.matmul(ps[:, 0:512], lhs, qT_sb[:, :, 0:512],
                                 start=True, stop=True, perf_mode=DR)
                nc.tensor.matmul(ps[:, 512:1024], lhs, qT_sb[:, :, 512:1024],
                                 start=True, stop=True, perf_mode=DR)
                if kind == "pool":
                    if half == 0:
                        state["pin"] = pin_pool.tile([128, 2048], f16,
                                                     name="pin_t", tag="pin")
                    pin = state["pin"]
                    nc.scalar.copy(out=pin[:, half * 1024:(half + 1) * 1024],
                                   in_=ps)
                    last = idx == PD - 1
                    if half == 1 or last:
                        pair = idx // 2
                        w = 1024 if (last and half == 0) else 2048
                        nc.gpsimd.partition_all_reduce(
                            pool_res[:, pair * 2048:pair * 2048 + w],
                            pin[:, 0:w], channels=128,
                            reduce_op=bass_isa.ReduceOp.max)
                        if pair == 5:
                            nc.sync.dma_start(
                                out=pool_out_d[0:6 * 2048],
                                in_=pool_res[0:1, 0:6 * 2048])
                else:
                    if half == 0:
                        state["shp"] = shp_pool.tile([128, 2048], f16,
                                                     name="shp_t", tag="shp")
                    sc = state["shp"]
                    nc.scalar.copy(out=sc[:, half * 1024:(half + 1) * 1024],
                                   in_=ps)
                    last = idx == SD - 1
                    if half == 1 or last:
                        nc.sync.dma_start(out=ship_out[idx // 2], in_=sc)

            n_dtiles = len(ddocs)
            for k in range(N_TCHUNK):
                q_chunk = qT_sb[:, :, k * 128:(k + 1) * 128]
                m_t = mt_pool.tile([128, TD], f32, name="m_t", tag="mt")
                for gi, (j0, n) in enumerate(tg):
                    ps = psum_pool.tile([128, 1024], f32, tag="ps")
                    ncols = n * D
                    i = 0
                    while i < ncols:
                        w = min(512, ncols - i)
                        nc.tensor.matmul(
                            ps[:, i:i + w], q_chunk,
                            qp_sb[:, :, T + j0 * D + i:T + j0 * D + i + w],
                            start=True, stop=True, perf_mode=DR)
                        i += w
                    nc.vector.tensor_reduce(
                        out=m_t[:, j0:j0 + n],
                        in_=ps[:, 0:ncols].rearrange("p (g d) -> p g d", d=D),
                        axis=mybir.AxisListType.X, op=mx)
                    if j0 + n >= TD:
                        nc.sync.dma_start(out=m_out[k], in_=m_t)
                    # interleave d-layout docs between T-tiles; finish them
                    # by ~7/8 of the chunks so pool/ship drain during the
                    # last chunk's DVE work
                    step = k * len(tg) + gi + 1
                    want = (n_dtiles * FRONT_NUM * step) // (
                        FRONT_DEN * len(tg) * N_TCHUNK)
                    while state["emitted"] < min(want, n_dtiles):
                        emit_ddoc()

            nc.sync.dma_start(out=pool_out_d[6 * 2048:],
                              in_=pool_res[0:1, 6 * 2048:])
    nc.compile()
    return nc


def _get_nc(mode: str = "float8dr") -> bass.Bass:
    if mode not in _NC_CACHE:
        _NC_CACHE[mode] = _build(mode)
    return _NC_CACHE[mode]


def kernel(query_embeddings, positive_embeddings):
    global LAST_RESULTS
    q = np.ascontiguousarray(np.asarray(query_embeddings, dtype=np.float32))
    p = np.ascontiguousarray(np.asarray(positive_embeddings, dtype=np.float32))
    assert q.shape == (B, S, H) and p.shape == (C, D, H)

    import ml_dtypes
    f8 = ml_dtypes.float8_e4m3
    # h split over [64 partitions, 2 k-subtiles]: h = 2*p + j
    qT8 = np.ascontiguousarray(
        q.reshape(T, H).T.reshape(H // 2, 2, T)).astype(f8)
    pT = p.transpose(2, 0, 1)                                  # [H, C, D] view
    in_maps = []
    for core in range(N_CORES):
        blk = pT[:, core * C_LOC:(core + 1) * C_LOC, :]        # [H, C_LOC, D]
        p8 = np.ascontiguousarray(
            blk.reshape(H // 2, 2, C_LOC * D)).astype(f8)
        in_maps.append({
            "qp": np.ascontiguousarray(np.concatenate([qT8, p8], axis=2)),
        })

    nc = _get_nc(MM_DTYPE)
    res = run_bass_kernel_spmd(
        nc, in_maps, core_ids=list(range(N_CORES)), trace=False)
    LAST_RESULTS = res

    m_parts = []
    for core, r in enumerate(res.results):
        m = np.empty((T, C_LOC), dtype=np.float32)
        # DVE lane: m_out [128, k, j] -> token = k*128 + p
        mt = r["m_out"]                                        # [k, 128, TD]
        for k in range(N_TCHUNK):
            m[k * 128:(k + 1) * 128, 0:TD] = mt[k]
        # pool lane: row [N_PPAIR, 2, 1024] -> doc TD + 2t+h, token col
        pr = r["pool_row"].astype(np.float32).reshape(N_PPAIR, 2, 1024)
        for j in range(PD):
            m[:, TD + j] = pr[j // 2, j % 2]
        # ship lane: [N_SPAIR, 128, 2048] -> max over d (axis 1)
        sh = r["ship_out"].astype(np.float32).max(axis=1)      # [N_SPAIR,2048]
        sh = sh.reshape(N_SPAIR, 2, 1024)
        for j in range(SD):
            m[:, TD + PD + j] = sh[j // 2, j % 2]
        m_parts.append(m)
    m_full = np.concatenate(m_parts, axis=1)                   # [T, C]

    screen = m_full.reshape(B, S, C).sum(axis=1, dtype=np.float64)
    screen /= TEMPERATURE                                      # [B, C]

    # exact rescore of top-K candidates (plus doc 0) per query, fp64
    loss_terms = np.empty(B, dtype=np.float64)
    q64 = q.astype(np.float64)
    p64 = p.astype(np.float64)
    def exact_scores(b, cand):
        late = np.einsum("sh,cdh->csd", q64[b], p64[cand])     # [k, S, D]
        return late.max(axis=2).sum(axis=1) / TEMPERATURE      # [k]

    for b in range(B):
        cand = np.argsort(screen[b])[-K_RESCORE:]
        cand = np.unique(np.concatenate([cand, [0]]))
        s_exact = exact_scores(b, cand)
        sb = screen[b].copy()
        sb[cand] = s_exact
        # guard: exact-rescore anything screened within 500 of the exact
        # candidate max (normally selects nothing; catches screen glitches)
        thresh = s_exact.max() - 500.0
        extra = np.setdiff1d(np.where(sb > thresh)[0], cand)
        if extra.size:
            sb[extra] = exact_scores(b, extra)
        mxv = sb.max()
        lse = mxv + np.log(np.exp(sb - mxv).sum())
        loss_terms[b] = lse - sb[0]
    loss = loss_terms.mean()
    return np.asarray(loss, dtype=np.float32)
